# revision 1
# baseline (speedup 1.0000x reference)
"""AffineLayer2d (random affine augmentation sampling) for 8 trn2 NeuronCores.

Strategy (data-parallel per sharding hint): shard batch N=8, one image per
core. Host computes the affine parameters (exact fp32 replica of the
reference's expm3) and the per-pixel corner gather (the irregular-index part
that profiled 10-50x too slow on every device gather path: SWDGE indirect DMA
is limited to 128 offsets/instruction on HW, ap_gather measured 27ns/idx,
dma_gather crashes above 1024 idx/instruction). The device kernel performs
the sharded bilinear weighting + blend (4 mul + 3 add per output element)
over each core's [32,3,224,224] shard and writes the output shard.
Falls back to the pure-NumPy path if the device toolchain is unavailable.
"""
import numpy as np

N, C, H, W = 8, 3, 224, 224
S = 32
PI = 3.141592653589793

_GENS = np.zeros((6, 3, 3), dtype=np.float32)
_GENS[0, 0, 2] = 1.0
_GENS[1, 1, 2] = 1.0
_GENS[2, 0, 1] = -1.0
_GENS[2, 1, 0] = 1.0
_GENS[3, 0, 0] = 1.0
_GENS[4, 1, 1] = 1.0
_GENS[5, 0, 1] = 1.0
_GENS[5, 1, 0] = 1.0


def _expm3(A):
    s = 6
    A = (A / np.float32(2.0 ** s)).astype(np.float32)
    I = np.eye(3, dtype=np.float32)
    out = (I + A).astype(np.float32)
    term = A.copy()
    for i in range(2, 13):
        term = (term @ A) / np.float32(i)
        out = out + term
    for _ in range(s):
        out = out @ out
    return out


def _corners_and_weights(x, ksamp, rot_factor):
    """Exact fp32 replica of the reference sampling math. Returns the four
    corner-value arrays and weight arrays for each (n,s) grid."""
    k = (ksamp.astype(np.float32) * np.float32(2.0) - np.float32(1.0))
    rf = rot_factor.astype(np.float32)
    coeff = np.array([rf[0], rf[1], np.clip(rf[2], -PI, PI), rf[3], rf[4], rf[5]],
                     dtype=np.float32)
    M = np.einsum('kns,k,kij->nsij', k, coeff, _GENS).astype(np.float32)
    theta = _expm3(M.reshape(N * S, 3, 3))[:, :2, :]          # [N*S,2,3]

    xs = np.linspace(-1.0, 1.0, W, dtype=np.float32)
    ys = np.linspace(-1.0, 1.0, H, dtype=np.float32)
    gx, gy = np.meshgrid(xs, ys)                               # [H,W]
    base = np.stack([gx, gy, np.ones_like(gx)], -1).astype(np.float32)  # [H,W,3]
    grid = np.einsum('bij,hwj->bhwi', theta, base).astype(np.float32)   # [B,H,W,2]

    ix = ((grid[..., 0] + np.float32(1.0)) * np.float32(0.5) * np.float32(W - 1)).astype(np.float32)
    iy = ((grid[..., 1] + np.float32(1.0)) * np.float32(0.5) * np.float32(H - 1)).astype(np.float32)
    x0 = np.floor(ix)
    y0 = np.floor(iy)
    wx1 = (ix - x0).astype(np.float32)
    wx0 = (np.float32(1.0) - wx1).astype(np.float32)
    wy1 = (iy - y0).astype(np.float32)
    wy0 = (np.float32(1.0) - wy1).astype(np.float32)

    def gather(img_ns, yf, xf):
        valid = ((xf >= 0) & (xf <= W - 1) & (yf >= 0) & (yf <= H - 1))
        xi = np.clip(xf, 0, W - 1).astype(np.int32)
        yi = np.clip(yf, 0, H - 1).astype(np.int32)
        b = np.arange(N * S)[:, None, None]
        vals = img_ns[b, :, yi, xi]                            # [B,H,W,C]
        vals = np.moveaxis(vals, -1, 1)                        # [B,C,H,W]
        return (vals * valid[:, None, :, :]).astype(np.float32)

    imgs = np.broadcast_to(x[:, None], (N, S, C, H, W)).reshape(N * S, C, H, W)
    c00 = gather(imgs, y0, x0)
    c01 = gather(imgs, y0, x0 + 1.0)
    c10 = gather(imgs, y0 + 1.0, x0)
    c11 = gather(imgs, y0 + 1.0, x0 + 1.0)
    w00 = (wy0 * wx0).astype(np.float32)                       # [B,H,W]
    w01 = (wy0 * wx1).astype(np.float32)
    w10 = (wy1 * wx0).astype(np.float32)
    w11 = (wy1 * wx1).astype(np.float32)
    return (c00, c01, c10, c11), (w00, w01, w10, w11)


def _blend_numpy(cs, ws):
    out = (cs[0] * ws[0][:, None] + cs[1] * ws[1][:, None]
           + cs[2] * ws[2][:, None] + cs[3] * ws[3][:, None])
    return out.reshape(N, S, C, H, W).astype(np.float32)


def _blend_device(cs, ws):
    import sys
    if '/opt/trn_rl_repo' not in sys.path:
        sys.path.insert(0, '/opt/trn_rl_repo')
    import concourse.bacc as bacc
    import concourse.mybir as mybir
    from concourse import tile
    from concourse.bass_utils import run_bass_kernel_spmd

    # Per-core shard: n-th image's S*C*H*W elements, flattened to [128, FREE].
    PER = S * C * H * W                   # 4,816,896 per core
    P = 128
    FREE = PER // P                       # 37,632
    CH = 3136                             # free-dim chunk (12 chunks)
    NCH = FREE // CH

    nc = bacc.Bacc("TRN2", target_bir_lowering=False, debug=False, num_devices=8)
    din = {}
    for nm in ("c00", "c01", "c10", "c11", "w00", "w01", "w10", "w11"):
        din[nm] = nc.dram_tensor(nm, [P, FREE], mybir.dt.float32, kind="ExternalInput")
    dout = nc.dram_tensor("out", [P, FREE], mybir.dt.float32, kind="ExternalOutput")

    with tile.TileContext(nc) as tc:
        with tc.tile_pool(name="p", bufs=2) as pool:
            for j in range(NCH):
                sl = slice(j * CH, (j + 1) * CH)
                acc = pool.tile([P, CH], mybir.dt.float32)
                tmp = pool.tile([P, CH], mybir.dt.float32)
                first = True
                for cn, wn in (("c00", "w00"), ("c01", "w01"),
                               ("c10", "w10"), ("c11", "w11")):
                    ct = pool.tile([P, CH], mybir.dt.float32, tag="ct")
                    wt = pool.tile([P, CH], mybir.dt.float32, tag="wt")
                    nc.sync.dma_start(out=ct[:, :], in_=din[cn][:, sl])
                    nc.sync.dma_start(out=wt[:, :], in_=din[wn][:, sl])
                    if first:
                        nc.vector.tensor_tensor(out=acc[:, :], in0=ct[:, :],
                                                in1=wt[:, :], op=mybir.AluOpType.mult)
                        first = False
                    else:
                        nc.vector.tensor_tensor(out=tmp[:, :], in0=ct[:, :],
                                                in1=wt[:, :], op=mybir.AluOpType.mult)
                        nc.vector.tensor_tensor(out=acc[:, :], in0=acc[:, :],
                                                in1=tmp[:, :], op=mybir.AluOpType.add)
                nc.sync.dma_start(out=dout[:, sl], in_=acc[:, :])
    nc.compile()

    # Build per-core input maps: core i gets image i's samples.
    in_maps = []
    wb = [np.broadcast_to(w[:, None], (N * S, C, H, W)) for w in
          (ws[0].reshape(N * S, 1, H, W)[:, 0], ws[1].reshape(N * S, 1, H, W)[:, 0],
           ws[2].reshape(N * S, 1, H, W)[:, 0], ws[3].reshape(N * S, 1, H, W)[:, 0])]
    for i in range(8):
        rows = slice(i * S, (i + 1) * S)
        m = {}
        for nm, arr in (("c00", cs[0]), ("c01", cs[1]), ("c10", cs[2]), ("c11", cs[3])):
            m[nm] = np.ascontiguousarray(arr[rows]).reshape(P, FREE)
        for nm, arr in (("w00", wb[0]), ("w01", wb[1]), ("w10", wb[2]), ("w11", wb[3])):
            m[nm] = np.ascontiguousarray(arr[rows]).reshape(P, FREE)
        in_maps.append(m)

    res = run_bass_kernel_spmd(nc, in_maps, core_ids=list(range(8)))
    out = np.empty((N, S, C, H, W), np.float32)
    for i in range(8):
        out[i] = res.results[i]["out"].reshape(S, C, H, W)
    return out


def kernel(x, ksamp, rot_factor):
    x = np.asarray(x, dtype=np.float32)
    ksamp = np.asarray(ksamp, dtype=np.float32)
    rot_factor = np.asarray(rot_factor, dtype=np.float32)
    cs, ws = _corners_and_weights(x, ksamp, rot_factor)
    try:
        return _blend_device(cs, ws)
    except Exception as e:  # device/toolchain unavailable -> numpy fallback
        import sys
        print(f"kernel.py: device path failed ({type(e).__name__}: {e}); "
              f"using numpy fallback", file=sys.stderr)
        return _blend_numpy(cs, ws)



# revision 2
# speedup vs baseline: 1.0525x; 1.0525x over previous
"""AffineLayer2d (random affine grid_sample) for 8 trn2 NeuronCores.

Data-parallel: core n handles image n (all S=32 samples). The bilinear
gather runs ON DEVICE via SWDGE dma_gather: the host uploads, per core, a
bf16 "pair-plane" PP[c, r, :] holding (img[r], img[r+1]) interleaved with
64-pair zero margins; the device expands it into 32 column-shifted copies
P[28800, 384] (one 768B gather-row = 3 channels x 64 pair-positions) so
that every output pixel's 4 bilinear corners for all 3 channels live in
ONE int16-addressable gather-row at offsets {128c + 64F + 0..3}. One
dma_gather index per output pixel (1.6M/core, 1024 idx/call), then the
vector engine applies host-computed folded weights (wlo = w*(1-F),
whi = w*F, zero-padded/validity-folded) and reduces 4->1.

Falls back to a pure-numpy path if the device toolchain is unavailable.
"""
import numpy as np

N, C, H, W, S = 8, 3, 224, 224, 32
PI = 3.141592653589793

NK, NR, NB = 32, 225, 4          # copies, P rows/copy, x-blocks/row
GROW, GE = NK * NR * NB, 384     # gather rows (28800), bf16 elems per row
QT, QP, QJ = 224, 128, 56        # tiles, partitions, x-cols per tile
QPX = QP * QJ                    # 7168 px per tile
NCALL, CALLI = 7, 1024           # gather calls per tile, idx per call
PHQ, NPH = 28, 8                 # tiles per idx phase, phases
PHW = QPX * PHQ // 16            # 12544 wrapped idx cols per phase

_GENS = np.zeros((6, 3, 3), dtype=np.float32)
_GENS[0, 0, 2] = 1.0
_GENS[1, 1, 2] = 1.0
_GENS[2, 0, 1] = -1.0
_GENS[2, 1, 0] = 1.0
_GENS[3, 0, 0] = 1.0
_GENS[4, 1, 1] = 1.0
_GENS[5, 0, 1] = 1.0
_GENS[5, 1, 0] = 1.0


def _expm3(A):
    s = 6
    A = (A / np.float32(2.0 ** s)).astype(np.float32)
    I = np.eye(3, dtype=np.float32)
    out = (I + A).astype(np.float32)
    term = A.copy()
    for i in range(2, 13):
        term = (term @ A) / np.float32(i)
        out = out + term
    for _ in range(s):
        out = out @ out
    return out


def _theta(ksamp, rot_factor):
    k = (ksamp.astype(np.float32) * np.float32(2.0) - np.float32(1.0))
    rf = rot_factor.astype(np.float32)
    coeff = np.array([rf[0], rf[1], np.clip(rf[2], -PI, PI), rf[3], rf[4], rf[5]],
                     dtype=np.float32)
    M = np.einsum('kns,k,kij->nsij', k, coeff, _GENS).astype(np.float32)
    return _expm3(M.reshape(N * S, 3, 3))[:, :2, :]          # [256,2,3]


def _grid_terms(x, ksamp, rot_factor):
    """Per-pixel sampling terms for all 256 (n,s) grids, f32 [256,H,W]."""
    th = _theta(ksamp, rot_factor)
    xs = np.linspace(-1.0, 1.0, W, dtype=np.float32)
    ys = np.linspace(-1.0, 1.0, H, dtype=np.float32)
    gx, gy = np.meshgrid(xs, ys)
    gx = gx.astype(np.float32)[None]
    gy = gy.astype(np.float32)[None]
    ix = ((th[:, 0, 0, None, None] * gx + th[:, 0, 1, None, None] * gy
           + th[:, 0, 2, None, None]) + np.float32(1.0)) * np.float32(0.5 * (W - 1))
    iy = ((th[:, 1, 0, None, None] * gx + th[:, 1, 1, None, None] * gy
           + th[:, 1, 2, None, None]) + np.float32(1.0)) * np.float32(0.5 * (H - 1))
    x0f = np.floor(ix)
    y0f = np.floor(iy)
    wx1 = (ix - x0f).astype(np.float32)
    wx0 = (np.float32(1.0) - wx1)
    wy1 = (iy - y0f).astype(np.float32)
    wy0 = (np.float32(1.0) - wy1)
    vx0 = (x0f >= 0) & (x0f <= W - 1)
    vx1 = (x0f >= -1) & (x0f <= W - 2)
    vy0 = (y0f >= 0) & (y0f <= H - 1)
    vy1 = (y0f >= -1) & (y0f <= H - 2)
    wA = wy0 * wx0 * (vy0 & vx0)
    wB = wy1 * wx0 * (vy1 & vx0)
    wC = wy0 * wx1 * (vy0 & vx1)
    wD = wy1 * wx1 * (vy1 & vx1)
    return x0f, y0f, wA, wB, wC, wD


def _host_precompute(x, ksamp, rot_factor):
    import ml_dtypes
    bf16 = ml_dtypes.bfloat16
    x0f, y0f, wA, wB, wC, wD = _grid_terms(x, ksamp, rot_factor)

    x0i = np.clip(x0f, -1, 224).astype(np.int32)
    y0i = np.clip(y0f, -1, 223).astype(np.int32)
    k = (-x0i) % NK
    p0 = x0i + k                              # in [0,255], = 0 mod 32
    r = y0i + 1                               # [0,224]
    b = p0 >> 6
    F = ((p0 >> 5) & 1).astype(np.float32)
    gidx = (k * (NR * NB) + r * NB + b).astype(np.int16)      # [256,H,W]

    w4 = np.stack([wA, wB, wC, wD], axis=-1)                   # [256,H,W,4]
    wlo = (w4 * (np.float32(1.0) - F)[..., None]).astype(bf16)
    whi = (w4 * F[..., None]).astype(bf16)

    PPs, IXs, WFs = [], [], []
    for n in range(N):
        imgpad = np.zeros((C, H + 2, W), np.float32)
        imgpad[:, 1:H + 1] = x[n]
        PP = np.zeros((C, NR, 704), bf16)
        PP[:, :, 128:576:2] = imgpad[:, 0:NR].astype(bf16)
        PP[:, :, 129:577:2] = imgpad[:, 1:NR + 1].astype(bf16)
        PPs.append(PP)

        gi = gidx[n * S:(n + 1) * S].reshape(QPX, W)           # [7168,224]
        T = gi.reshape(56, QP, 4, QJ)                          # u p v j
        st = T.transpose(0, 2, 3, 1).reshape(QT * QPX)         # ((u v) (j p))
        IXs.append(np.ascontiguousarray(st.reshape(-1, 16).T)) # [16,100352]

        lo = wlo[n * S:(n + 1) * S].reshape(QPX, W, 4).reshape(56, QP, 4, QJ, 4)
        hi = whi[n * S:(n + 1) * S].reshape(QPX, W, 4).reshape(56, QP, 4, QJ, 4)
        pk = np.concatenate([lo, hi], axis=-1)                 # [u,p,v,j,8]
        WFs.append(np.ascontiguousarray(
            pk.transpose(0, 2, 1, 3, 4).reshape(QT * QP, QJ * 8)))
    return PPs, IXs, WFs


def _reorder_out(dev):
    """[224*128,168] f32 -> [S,C,H,W]."""
    R = dev.reshape(56, 4, QP, C, QJ).transpose(3, 0, 2, 1, 4)
    return np.ascontiguousarray(
        R.reshape(C, QPX, W).reshape(C, S, H, W).transpose(1, 0, 2, 3))


def _numpy_fallback(x, ksamp, rot_factor):
    x0f, y0f, wA, wB, wC, wD = _grid_terms(x, ksamp, rot_factor)
    xi0 = np.clip(x0f, 0, W - 1).astype(np.int32)
    xi1 = np.clip(x0f + 1, 0, W - 1).astype(np.int32)
    yi0 = np.clip(y0f, 0, H - 1).astype(np.int32)
    yi1 = np.clip(y0f + 1, 0, H - 1).astype(np.int32)
    bn = (np.arange(N * S) // S)[:, None, None]
    out = np.empty((N * S, C, H, W), np.float32)
    for corner, (yi, xi, w) in enumerate(
            [(yi0, xi0, wA), (yi1, xi0, wB), (yi0, xi1, wC), (yi1, xi1, wD)]):
        v = x[bn, :, yi, xi]                                   # [B,H,W,C]
        v = np.moveaxis(v, -1, 1) * w[:, None]
        out = v if corner == 0 else out + v
    return out.reshape(N, S, C, H, W).astype(np.float32)


def _build_nc():
    import sys
    if '/opt/trn_rl_repo' not in sys.path:
        sys.path.insert(0, '/opt/trn_rl_repo')
    import concourse.bacc as bacc
    import concourse.mybir as mybir
    from concourse.library_config import mlp

    nc = bacc.Bacc("TRN2", target_bir_lowering=False, debug=False, num_devices=8)
    PP = nc.dram_tensor("PP", [C, NR, 704], mybir.dt.bfloat16, kind="ExternalInput")
    IX = nc.dram_tensor("IX", [16, QT * QPX // 16], mybir.dt.int16,
                        kind="ExternalInput")
    WF = nc.dram_tensor("WF", [QT * QP, QJ * 8], mybir.dt.bfloat16,
                        kind="ExternalInput")
    P = nc.dram_tensor("P", [GROW, GE], mybir.dt.bfloat16, kind="Internal")
    OUT = nc.dram_tensor("OUT", [QT * QP, C * QJ], mybir.dt.float32,
                         kind="ExternalOutput")

    with (
        nc.Block() as block,
        nc.sbuf_tensor("dst", [QP, 2, QJ, GE], mybir.dt.bfloat16) as dst,
        nc.sbuf_tensor("idx", [QP, 2, PHW], mybir.dt.int16) as idx,
        nc.sbuf_tensor("w", [QP, 2, QJ, 8], mybir.dt.bfloat16) as w,
        nc.sbuf_tensor("s1", [QP, 2, QJ, 4], mybir.dt.float32) as s1,
        nc.sbuf_tensor("s2", [QP, 2, QJ, 4], mybir.dt.float32) as s2,
        nc.sbuf_tensor("ob", [QP, 2, C * QJ], mybir.dt.float32) as ob,
        nc.semaphore("sP") as sP,
        nc.semaphore("sI") as sI,
        nc.semaphore("sW") as sW,
        nc.semaphore("sG") as sG,
        nc.semaphore("sB") as sB,
        nc.semaphore("sO") as sO,
    ):
        @block.sync
        def _(sync):
            for k in range(NK):
                off = 128 - 2 * k
                src = PP[:, :, off:off + 512].rearrange("c r (b e) -> c r b e", b=NB)
                dstap = P[k * NR * NB:(k + 1) * NR * NB, :].rearrange(
                    "(r b) (c e) -> c r b e", b=NB, c=C)
                sync.dma_start(dstap, src).then_inc(sP, 16)
            for ph in range(2):
                for g in range(8):
                    sync.dma_start(idx[16 * g:16 * g + 16, ph, :],
                                   IX[:, ph * PHW:(ph + 1) * PHW]).then_inc(sI, 16)
            for t in range(QT):
                if t % PHQ == 0 and t // PHQ + 2 < NPH:
                    ph = t // PHQ + 2
                    sync.wait_ge(sG, 16 * NCALL * (ph - 1) * PHQ)
                    for g in range(8):
                        sync.dma_start(idx[16 * g:16 * g + 16, ph % 2, :],
                                       IX[:, ph * PHW:(ph + 1) * PHW]).then_inc(sI, 16)
                if t >= 2:
                    sync.wait_ge(sB, t - 1)
                sync.dma_start(
                    w[:, t % 2, :, :],
                    WF[t * QP:(t + 1) * QP, :].rearrange("p (j o) -> p j o", o=8)
                ).then_inc(sW, 16)
            sync.wait_ge(sI, 16 * 8 * NPH)
            sync.wait_ge(sW, 16 * QT)
            sync.wait_ge(sP, 16 * NK)

        @block.gpsimd
        def _(gp):
            gp.load_library(mlp)
            gp.wait_ge(sP, 16 * NK)
            for t in range(QT):
                ph = t // PHQ
                gp.wait_ge(sI, 16 * 8 * (ph + 1))
                if t >= 2:
                    gp.wait_ge(sB, t - 1)
                for c8 in range(NCALL):
                    col = (t % PHQ) * (QPX // 16) + c8 * (CALLI // 16)
                    gp.dma_gather(
                        dst[:, t % 2, 8 * c8:8 * c8 + 8, :], P[:, :],
                        idx[:, ph % 2, col:col + CALLI // 16],
                        CALLI, CALLI, GE).then_inc(sG, 16)
            gp.wait_ge(sG, 16 * NCALL * QT)

        @block.vector
        def _(ve):
            for t in range(QT):
                ve.wait_ge(sW, 16 * (t + 1))
                ve.wait_ge(sG, 16 * NCALL * (t + 1))
                if t >= 2:
                    ve.wait_ge(sO, 16 * (t - 1))
                z = t % 2
                for c in range(C):
                    glo = dst[:, z, :, 128 * c:128 * c + 4]
                    ghi = dst[:, z, :, 128 * c + 64:128 * c + 68]
                    ve.tensor_tensor(out=s1[:, z, :, :], in0=glo, in1=w[:, z, :, 0:4],
                                     op=mybir.AluOpType.mult)
                    ve.tensor_tensor(out=s2[:, z, :, :], in0=ghi, in1=w[:, z, :, 4:8],
                                     op=mybir.AluOpType.mult)
                    ve.tensor_tensor(out=s1[:, z, :, :], in0=s1[:, z, :, :],
                                     in1=s2[:, z, :, :], op=mybir.AluOpType.add)
                    red = ve.tensor_reduce(out=ob[:, z, QJ * c:QJ * c + QJ],
                                           in_=s1[:, z, :, :],
                                           axis=mybir.AxisListType.X,
                                           op=mybir.AluOpType.add)
                red.then_inc(sB, 1)

        @block.scalar
        def _(sc):
            for t in range(QT):
                sc.wait_ge(sB, t + 1)
                sc.dma_start(OUT[t * QP:(t + 1) * QP, :],
                             ob[:, t % 2, :]).then_inc(sO, 16)
            sc.wait_ge(sO, 16 * QT)

    nc.compile()
    return nc


def kernel(x, ksamp, rot_factor):
    import sys, time
    x = np.asarray(x, dtype=np.float32)
    ksamp = np.asarray(ksamp, dtype=np.float32)
    rot_factor = np.asarray(rot_factor, dtype=np.float32)
    try:
        PPs, IXs, WFs = _host_precompute(x, ksamp, rot_factor)
        nc = _build_nc()
        if '/opt/trn_rl_repo' not in sys.path:
            sys.path.insert(0, '/opt/trn_rl_repo')
        from concourse.bass_utils import run_bass_kernel_spmd
        in_maps = [{"PP": PPs[n], "IX": IXs[n], "WF": WFs[n]} for n in range(N)]
        res = None
        for attempt in range(3):
            try:
                res = run_bass_kernel_spmd(nc, in_maps, core_ids=list(range(8)))
                break
            except Exception as e:
                print(f"kernel.py: device attempt {attempt} failed "
                      f"({type(e).__name__}); retrying", file=sys.stderr)
                time.sleep(5)
        if res is None:
            raise RuntimeError("device retries exhausted")
        out = np.empty((N, S, C, H, W), np.float32)
        for n in range(N):
            out[n] = _reorder_out(res.results[n]["OUT"])
        return out
    except Exception as e:
        print(f"kernel.py: device path failed ({type(e).__name__}: {e}); "
              f"using numpy fallback", file=sys.stderr)
        return _numpy_fallback(x, ksamp, rot_factor)


# revision 5
# speedup vs baseline: 4.3142x; 4.0989x over previous
"""AffineLayer2d (random affine grid_sample) for 8 trn2 NeuronCores.

Data-parallel: core n handles image n (all S=32 samples). The bilinear
gather runs ON DEVICE via SWDGE dma_gather: the host uploads, per core, a
bf16 "pair-plane" PP[c, r, :] holding (img[r], img[r+1]) interleaved with
64-pair zero margins; the device expands it into 32 column-shifted copies
P[28800, 384] (one 768B gather-row = 3 channels x 64 pair-positions) so
that every output pixel's 4 bilinear corners for all 3 channels live in
ONE int16-addressable gather-row at offsets {128c + 64F + 0..3}. One
dma_gather index per output pixel (1.6M/core, 1024 idx/call), then the
vector engine applies host-computed folded weights (wlo = w*(1-F),
whi = w*F, zero-padded/validity-folded) and reduces 4->1.

Falls back to a pure-numpy path if the device toolchain is unavailable.
"""
import numpy as np

N, C, H, W, S = 8, 3, 224, 224, 32
PI = 3.141592653589793

NK, NR, NB = 32, 225, 4          # copies, P rows/copy, x-blocks/row
GROW, GE = NK * NR * NB, 384     # gather rows (28800), bf16 elems per row
QT, QP, QJ = 224, 128, 56        # tiles, partitions, x-cols per tile
QPX = QP * QJ                    # 7168 px per tile
NCALL, CALLI = 7, 1024           # gather calls per tile, idx per call
PHQ, NPH = 28, 8                 # tiles per idx phase, phases
PHW = QPX * PHQ // 16            # 12544 wrapped idx cols per phase

_GENS = np.zeros((6, 3, 3), dtype=np.float32)
_GENS[0, 0, 2] = 1.0
_GENS[1, 1, 2] = 1.0
_GENS[2, 0, 1] = -1.0
_GENS[2, 1, 0] = 1.0
_GENS[3, 0, 0] = 1.0
_GENS[4, 1, 1] = 1.0
_GENS[5, 0, 1] = 1.0
_GENS[5, 1, 0] = 1.0


def _expm3(A):
    s = 6
    A = (A / np.float32(2.0 ** s)).astype(np.float32)
    I = np.eye(3, dtype=np.float32)
    out = (I + A).astype(np.float32)
    term = A.copy()
    for i in range(2, 13):
        term = (term @ A) / np.float32(i)
        out = out + term
    for _ in range(s):
        out = out @ out
    return out


def _theta(ksamp, rot_factor):
    k = (ksamp.astype(np.float32) * np.float32(2.0) - np.float32(1.0))
    rf = rot_factor.astype(np.float32)
    coeff = np.array([rf[0], rf[1], np.clip(rf[2], -PI, PI), rf[3], rf[4], rf[5]],
                     dtype=np.float32)
    M = np.einsum('kns,k,kij->nsij', k, coeff, _GENS).astype(np.float32)
    return _expm3(M.reshape(N * S, 3, 3))[:, :2, :]          # [256,2,3]


def _grid_terms(x, ksamp, rot_factor):
    """Per-pixel sampling terms for all 256 (n,s) grids, f32 [256,H,W]."""
    th = _theta(ksamp, rot_factor)
    xs = np.linspace(-1.0, 1.0, W, dtype=np.float32)
    ys = np.linspace(-1.0, 1.0, H, dtype=np.float32)
    gx, gy = np.meshgrid(xs, ys)
    gx = gx.astype(np.float32)[None]
    gy = gy.astype(np.float32)[None]
    ix = ((th[:, 0, 0, None, None] * gx + th[:, 0, 1, None, None] * gy
           + th[:, 0, 2, None, None]) + np.float32(1.0)) * np.float32(0.5 * (W - 1))
    iy = ((th[:, 1, 0, None, None] * gx + th[:, 1, 1, None, None] * gy
           + th[:, 1, 2, None, None]) + np.float32(1.0)) * np.float32(0.5 * (H - 1))
    x0f = np.floor(ix)
    y0f = np.floor(iy)
    wx1 = (ix - x0f).astype(np.float32)
    wx0 = (np.float32(1.0) - wx1)
    wy1 = (iy - y0f).astype(np.float32)
    wy0 = (np.float32(1.0) - wy1)
    vx0 = (x0f >= 0) & (x0f <= W - 1)
    vx1 = (x0f >= -1) & (x0f <= W - 2)
    vy0 = (y0f >= 0) & (y0f <= H - 1)
    vy1 = (y0f >= -1) & (y0f <= H - 2)
    wA = wy0 * wx0 * (vy0 & vx0)
    wB = wy1 * wx0 * (vy1 & vx0)
    wC = wy0 * wx1 * (vy0 & vx1)
    wD = wy1 * wx1 * (vy1 & vx1)
    return x0f, y0f, wA, wB, wC, wD


def _host_precompute(x, ksamp, rot_factor):
    import ml_dtypes
    bf16 = ml_dtypes.bfloat16
    x0f, y0f, wA, wB, wC, wD = _grid_terms(x, ksamp, rot_factor)

    x0i = np.clip(x0f, -1, 224).astype(np.int32)
    y0i = np.clip(y0f, -1, 223).astype(np.int32)
    k = (-x0i) % NK
    p0 = x0i + k                              # in [0,255], = 0 mod 32
    r = y0i + 1                               # [0,224]
    b = p0 >> 6
    F = ((p0 >> 5) & 1).astype(np.float32)
    gidx = (k * (NR * NB) + r * NB + b).astype(np.int16)      # [256,H,W]

    w4 = np.stack([wA, wB, wC, wD], axis=-1)                   # [256,H,W,4]
    wlo = (w4 * (np.float32(1.0) - F)[..., None]).astype(bf16)
    whi = (w4 * F[..., None]).astype(bf16)

    PPs, IXs, WFs = [], [], []
    for n in range(N):
        imgpad = np.zeros((C, H + 2, W), np.float32)
        imgpad[:, 1:H + 1] = x[n]
        PP = np.zeros((C, NR, 704), bf16)
        PP[:, :, 128:576:2] = imgpad[:, 0:NR].astype(bf16)
        PP[:, :, 129:577:2] = imgpad[:, 1:NR + 1].astype(bf16)
        PPs.append(PP)

        gi = gidx[n * S:(n + 1) * S].reshape(QPX, W)           # [7168,224]
        T = gi.reshape(56, QP, 4, QJ)                          # u p v j
        st = T.transpose(0, 2, 3, 1).reshape(QT * QPX)         # ((u v) (j p))
        IXs.append(np.ascontiguousarray(st.reshape(-1, 16).T)) # [16,100352]

        lo = wlo[n * S:(n + 1) * S].reshape(QPX, W, 4).reshape(56, QP, 4, QJ, 4)
        hi = whi[n * S:(n + 1) * S].reshape(QPX, W, 4).reshape(56, QP, 4, QJ, 4)
        pk = np.concatenate([lo, hi], axis=-1)                 # [u,p,v,j,8]
        WFs.append(np.ascontiguousarray(
            pk.transpose(0, 2, 1, 3, 4).reshape(QT * QP, QJ * 8)))
    return PPs, IXs, WFs


def _reorder_out(dev):
    """[224*128,168] f32 -> [S,C,H,W]."""
    R = dev.reshape(56, 4, QP, C, QJ).transpose(3, 0, 2, 1, 4)
    return np.ascontiguousarray(
        R.reshape(C, QPX, W).reshape(C, S, H, W).transpose(1, 0, 2, 3))


def _numpy_fallback(x, ksamp, rot_factor):
    x0f, y0f, wA, wB, wC, wD = _grid_terms(x, ksamp, rot_factor)
    xi0 = np.clip(x0f, 0, W - 1).astype(np.int32)
    xi1 = np.clip(x0f + 1, 0, W - 1).astype(np.int32)
    yi0 = np.clip(y0f, 0, H - 1).astype(np.int32)
    yi1 = np.clip(y0f + 1, 0, H - 1).astype(np.int32)
    bn = (np.arange(N * S) // S)[:, None, None]
    out = np.empty((N * S, C, H, W), np.float32)
    for corner, (yi, xi, w) in enumerate(
            [(yi0, xi0, wA), (yi1, xi0, wB), (yi0, xi1, wC), (yi1, xi1, wD)]):
        v = x[bn, :, yi, xi]                                   # [B,H,W,C]
        v = np.moveaxis(v, -1, 1) * w[:, None]
        out = v if corner == 0 else out + v
    return out.reshape(N, S, C, H, W).astype(np.float32)


def _build_nc():
    import sys
    if '/opt/trn_rl_repo' not in sys.path:
        sys.path.insert(0, '/opt/trn_rl_repo')
    import concourse.bacc as bacc
    import concourse.mybir as mybir
    from concourse.library_config import mlp

    nc = bacc.Bacc("TRN2", target_bir_lowering=False, debug=False, num_devices=8)
    PP = nc.dram_tensor("PP", [C, NR, 704], mybir.dt.bfloat16, kind="ExternalInput")
    IX = nc.dram_tensor("IX", [16, QT * QPX // 16], mybir.dt.int16,
                        kind="ExternalInput")
    WF = nc.dram_tensor("WF", [QT * QP, QJ * 8], mybir.dt.bfloat16,
                        kind="ExternalInput")
    P = nc.dram_tensor("P", [GROW, GE], mybir.dt.bfloat16, kind="Internal")
    OUT = nc.dram_tensor("OUT", [QT * QP, C * QJ], mybir.dt.float32,
                         kind="ExternalOutput")

    with (
        nc.Block() as block,
        nc.sbuf_tensor("dst", [QP, 2, QJ, GE], mybir.dt.bfloat16) as dst,
        nc.sbuf_tensor("idx", [QP, 2, PHW], mybir.dt.int16) as idx,
        nc.sbuf_tensor("w", [QP, 2, QJ, 8], mybir.dt.bfloat16) as w,
        nc.sbuf_tensor("s1", [QP, 2, QJ, 4], mybir.dt.float32) as s1,
        nc.sbuf_tensor("s2", [QP, 2, QJ, 4], mybir.dt.float32) as s2,
        nc.sbuf_tensor("ob", [QP, 2, C * QJ], mybir.dt.float32) as ob,
        nc.semaphore("sP") as sP,
        nc.semaphore("sI0") as sI0,
        nc.semaphore("sI1") as sI1,
        nc.semaphore("sW0") as sW0,
        nc.semaphore("sW1") as sW1,
        nc.semaphore("sG0") as sG0,
        nc.semaphore("sG1") as sG1,
        nc.semaphore("sB") as sB,
        nc.semaphore("sO0") as sO0,
        nc.semaphore("sO1") as sO1,
    ):
        # DMA completions are NOT ordered across in-flight transfers, so a
        # single cumulative sem per stage can be satisfied by tile t+1's fast
        # DMAs while tile t's slow one is still landing. Per-slot sems make
        # each threshold count exactly the tiles sharing that buffer slot.
        sI = [sI0, sI1]
        sW = [sW0, sW1]
        sG = [sG0, sG1]
        sO = [sO0, sO1]
        @block.sync
        def _(sync):
            for k in range(NK):
                off = 128 - 2 * k
                src = PP[:, :, off:off + 512].rearrange("c r (b e) -> c r b e", b=NB)
                dstap = P[k * NR * NB:(k + 1) * NR * NB, :].rearrange(
                    "(r b) (c e) -> c r b e", b=NB, c=C)
                sync.dma_start(dstap, src).then_inc(sP, 16)
            for ph in range(2):
                for g in range(8):
                    sync.dma_start(idx[16 * g:16 * g + 16, ph, :],
                                   IX[:, ph * PHW:(ph + 1) * PHW]).then_inc(sI[ph], 16)
            for t in range(QT):
                if t >= 2:
                    sync.wait_ge(sB, t - 1)
                # refill idx phase ph at t=(ph-1)*PHQ+1: the sB wait above
                # implies all phase-(ph-2) tiles are gathered (slot free)
                if t >= PHQ + 1 and (t - 1) % PHQ == 0:
                    ph = (t - 1) // PHQ + 1
                    if ph < NPH:
                        for g in range(8):
                            sync.dma_start(idx[16 * g:16 * g + 16, ph % 2, :],
                                           IX[:, ph * PHW:(ph + 1) * PHW]
                                           ).then_inc(sI[ph % 2], 16)
                sync.dma_start(
                    w[:, t % 2, :, :],
                    WF[t * QP:(t + 1) * QP, :].rearrange("p (j o) -> p j o", o=8)
                ).then_inc(sW[t % 2], 16)
            for z in range(2):
                sync.wait_ge(sI[z], 16 * 8 * (NPH // 2))
                sync.wait_ge(sW[z], 16 * (QT // 2))
            sync.wait_ge(sP, 16 * NK)

        @block.gpsimd
        def _(gp):
            gp.load_library(mlp)
            gp.wait_ge(sP, 16 * NK)
            for t in range(QT):
                ph = t // PHQ
                gp.wait_ge(sI[ph % 2], 16 * 8 * (ph // 2 + 1))
                if t >= 2:
                    gp.wait_ge(sB, t - 1)
                for c8 in range(NCALL):
                    col = (t % PHQ) * (QPX // 16) + c8 * (CALLI // 16)
                    gp.dma_gather(
                        dst[:, t % 2, 8 * c8:8 * c8 + 8, :], P[:, :],
                        idx[:, ph % 2, col:col + CALLI // 16],
                        CALLI, CALLI, GE).then_inc(sG[t % 2], 16)
            for z in range(2):
                gp.wait_ge(sG[z], 16 * NCALL * (QT // 2))

        @block.vector
        def _(ve):
            for t in range(QT):
                ve.wait_ge(sW[t % 2], 16 * (t // 2 + 1))
                ve.wait_ge(sG[t % 2], 16 * NCALL * (t // 2 + 1))
                if t >= 2:
                    ve.wait_ge(sO[t % 2], 16 * (t // 2))
                z = t % 2
                for c in range(C):
                    glo = dst[:, z, :, 128 * c:128 * c + 4]
                    ghi = dst[:, z, :, 128 * c + 64:128 * c + 68]
                    ve.tensor_tensor(out=s1[:, z, :, :], in0=glo, in1=w[:, z, :, 0:4],
                                     op=mybir.AluOpType.mult)
                    ve.tensor_tensor(out=s2[:, z, :, :], in0=ghi, in1=w[:, z, :, 4:8],
                                     op=mybir.AluOpType.mult)
                    ve.tensor_tensor(out=s1[:, z, :, :], in0=s1[:, z, :, :],
                                     in1=s2[:, z, :, :], op=mybir.AluOpType.add)
                    red = ve.tensor_reduce(out=ob[:, z, QJ * c:QJ * c + QJ],
                                           in_=s1[:, z, :, :],
                                           axis=mybir.AxisListType.X,
                                           op=mybir.AluOpType.add)
                red.then_inc(sB, 1)

        @block.scalar
        def _(sc):
            for t in range(QT):
                sc.wait_ge(sB, t + 1)
                sc.dma_start(OUT[t * QP:(t + 1) * QP, :],
                             ob[:, t % 2, :]).then_inc(sO[t % 2], 16)
            for z in range(2):
                sc.wait_ge(sO[z], 16 * (QT // 2))

    nc.compile()
    return nc


def kernel(x, ksamp, rot_factor):
    import sys, time
    x = np.asarray(x, dtype=np.float32)
    ksamp = np.asarray(ksamp, dtype=np.float32)
    rot_factor = np.asarray(rot_factor, dtype=np.float32)
    try:
        PPs, IXs, WFs = _host_precompute(x, ksamp, rot_factor)
        nc = _build_nc()
        if '/opt/trn_rl_repo' not in sys.path:
            sys.path.insert(0, '/opt/trn_rl_repo')
        from concourse.bass_utils import run_bass_kernel_spmd
        in_maps = [{"PP": PPs[n], "IX": IXs[n], "WF": WFs[n]} for n in range(N)]
        res = None
        for attempt in range(3):
            try:
                res = run_bass_kernel_spmd(nc, in_maps, core_ids=list(range(8)))
                break
            except Exception as e:
                print(f"kernel.py: device attempt {attempt} failed "
                      f"({type(e).__name__}); retrying", file=sys.stderr)
                time.sleep(5)
        if res is None:
            raise RuntimeError("device retries exhausted")
        out = np.empty((N, S, C, H, W), np.float32)
        for n in range(N):
            out[n] = _reorder_out(res.results[n]["OUT"])
        return out
    except Exception as e:
        print(f"kernel.py: device path failed ({type(e).__name__}: {e}); "
              f"using numpy fallback", file=sys.stderr)
        return _numpy_fallback(x, ksamp, rot_factor)


# revision 7
# speedup vs baseline: 6.5744x; 1.5239x over previous
"""AffineLayer2d (random affine grid_sample) for 8 trn2 NeuronCores.

Data-parallel: core n handles image n (all S=32 samples). The bilinear
gather runs ON DEVICE via SWDGE dma_gather: the host uploads, per core, a
bf16 "pair-plane" PP[c, r, :] holding (img[r], img[r+1]) interleaved with
64-pair zero margins; the device expands it into 32 column-shifted copies
P[28800, 384] (one 768B gather-row = 3 channels x 64 pair-positions) so
that every output pixel's 4 bilinear corners for all 3 channels live in
ONE int16-addressable gather-row at offsets {128c + 64F + 0..3}. One
dma_gather index per output pixel (1.6M/core, 1024 idx/call), then the
vector engine applies host-computed folded weights (wlo = w*(1-F),
whi = w*F, zero-padded/validity-folded) and reduces 4->1.

Falls back to a pure-numpy path if the device toolchain is unavailable.
"""
import numpy as np

N, C, H, W, S = 8, 3, 224, 224, 32
PI = 3.141592653589793

NK, NR, NB = 32, 225, 4          # copies, P rows/copy, x-blocks/row
GROW, GE = NK * NR * NB, 384     # gather rows (28800), bf16 elems per row
QT, QP, QJ = 224, 128, 56        # tiles, partitions, x-cols per tile
QPX = QP * QJ                    # 7168 px per tile
NCALL, CALLI = 7, 1024           # gather calls per tile, idx per call
PHQ, NPH = 28, 8                 # tiles per idx phase, phases
PHW = QPX * PHQ // 16            # 12544 wrapped idx cols per phase

_GENS = np.zeros((6, 3, 3), dtype=np.float32)
_GENS[0, 0, 2] = 1.0
_GENS[1, 1, 2] = 1.0
_GENS[2, 0, 1] = -1.0
_GENS[2, 1, 0] = 1.0
_GENS[3, 0, 0] = 1.0
_GENS[4, 1, 1] = 1.0
_GENS[5, 0, 1] = 1.0
_GENS[5, 1, 0] = 1.0


def _expm3(A):
    s = 6
    A = (A / np.float32(2.0 ** s)).astype(np.float32)
    I = np.eye(3, dtype=np.float32)
    out = (I + A).astype(np.float32)
    term = A.copy()
    for i in range(2, 13):
        term = (term @ A) / np.float32(i)
        out = out + term
    for _ in range(s):
        out = out @ out
    return out


def _theta(ksamp, rot_factor):
    k = (ksamp.astype(np.float32) * np.float32(2.0) - np.float32(1.0))
    rf = rot_factor.astype(np.float32)
    coeff = np.array([rf[0], rf[1], np.clip(rf[2], -PI, PI), rf[3], rf[4], rf[5]],
                     dtype=np.float32)
    M = np.einsum('kns,k,kij->nsij', k, coeff, _GENS).astype(np.float32)
    return _expm3(M.reshape(N * S, 3, 3))[:, :2, :]          # [256,2,3]


def _grid_terms(x, ksamp, rot_factor):
    """Per-pixel sampling terms for all 256 (n,s) grids, f32 [256,H,W]."""
    th = _theta(ksamp, rot_factor)
    xs = np.linspace(-1.0, 1.0, W, dtype=np.float32)
    ys = np.linspace(-1.0, 1.0, H, dtype=np.float32)
    gx, gy = np.meshgrid(xs, ys)
    gx = gx.astype(np.float32)[None]
    gy = gy.astype(np.float32)[None]
    ix = ((th[:, 0, 0, None, None] * gx + th[:, 0, 1, None, None] * gy
           + th[:, 0, 2, None, None]) + np.float32(1.0)) * np.float32(0.5 * (W - 1))
    iy = ((th[:, 1, 0, None, None] * gx + th[:, 1, 1, None, None] * gy
           + th[:, 1, 2, None, None]) + np.float32(1.0)) * np.float32(0.5 * (H - 1))
    x0f = np.floor(ix)
    y0f = np.floor(iy)
    wx1 = (ix - x0f).astype(np.float32)
    wx0 = (np.float32(1.0) - wx1)
    wy1 = (iy - y0f).astype(np.float32)
    wy0 = (np.float32(1.0) - wy1)
    vx0 = (x0f >= 0) & (x0f <= W - 1)
    vx1 = (x0f >= -1) & (x0f <= W - 2)
    vy0 = (y0f >= 0) & (y0f <= H - 1)
    vy1 = (y0f >= -1) & (y0f <= H - 2)
    wA = wy0 * wx0 * (vy0 & vx0)
    wB = wy1 * wx0 * (vy1 & vx0)
    wC = wy0 * wx1 * (vy0 & vx1)
    wD = wy1 * wx1 * (vy1 & vx1)
    return x0f, y0f, wA, wB, wC, wD


def _bf16_round(a):
    """Fast f32 -> bf16 round-to-nearest-even (ml_dtypes astype is slow)."""
    import ml_dtypes
    u = np.ascontiguousarray(a, np.float32).view(np.uint32)
    r = ((u + np.uint32(0x7FFF) + ((u >> np.uint32(16)) & np.uint32(1)))
         >> np.uint32(16)).astype(np.uint16)
    return r.view(ml_dtypes.bfloat16)


def _host_precompute(x, ksamp, rot_factor):
    import ml_dtypes
    bf16 = ml_dtypes.bfloat16
    x0f, y0f, wA, wB, wC, wD = _grid_terms(x, ksamp, rot_factor)

    x0i = np.clip(x0f, -1, 224).astype(np.int32)
    y0i = np.clip(y0f, -1, 223).astype(np.int32)
    k = (-x0i) % NK
    p0 = x0i + k                              # in [0,255], = 0 mod 32
    r = y0i + 1                               # [0,224]
    b = p0 >> 6
    F = ((p0 >> 5) & 1).astype(np.float32)
    gidx = (k * (NR * NB) + r * NB + b).astype(np.int16)      # [256,H,W]

    w4 = _bf16_round(np.stack([wA, wB, wC, wD], axis=-1))      # [256,H,W,4]
    Fb = _bf16_round(F)                                         # exact {0,1}

    PPs, IXs, WFs = [], [], []
    for n in range(N):
        imgpad = np.zeros((C, H + 2, W), np.float32)
        imgpad[:, 1:H + 1] = x[n]
        PP = np.zeros((C, NR, 704), bf16)
        PP[:, :, 128:576:2] = imgpad[:, 0:NR].astype(bf16)
        PP[:, :, 129:577:2] = imgpad[:, 1:NR + 1].astype(bf16)
        PPs.append(PP)

        gi = gidx[n * S:(n + 1) * S].reshape(QPX, W)           # [7168,224]
        T = gi.reshape(56, QP, 4, QJ)                          # u p v j
        st = T.transpose(0, 2, 3, 1).reshape(QT * QPX)         # ((u v) (j p))
        IXs.append(np.ascontiguousarray(st.reshape(-1, 16).T)) # [16,100352]

        w4n = w4[n * S:(n + 1) * S].reshape(QPX, W, 4).reshape(56, QP, 4, QJ * 4)
        fn = Fb[n * S:(n + 1) * S].reshape(QPX, W).reshape(56, QP, 4, QJ)
        pk = np.concatenate([w4n, fn], axis=-1)                # [u,p,v,280]
        WFs.append(np.ascontiguousarray(
            pk.transpose(0, 2, 1, 3).reshape(QT * QP, QJ * 5)))
    return PPs, IXs, WFs


def _reorder_out(dev):
    """[224*128,168] bf16 -> [S,C,H,W] f32."""
    dev = dev.astype(np.float32)
    R = dev.reshape(56, 4, QP, C, QJ).transpose(3, 0, 2, 1, 4)
    return np.ascontiguousarray(
        R.reshape(C, QPX, W).reshape(C, S, H, W).transpose(1, 0, 2, 3))


def _numpy_fallback(x, ksamp, rot_factor):
    x0f, y0f, wA, wB, wC, wD = _grid_terms(x, ksamp, rot_factor)
    xi0 = np.clip(x0f, 0, W - 1).astype(np.int32)
    xi1 = np.clip(x0f + 1, 0, W - 1).astype(np.int32)
    yi0 = np.clip(y0f, 0, H - 1).astype(np.int32)
    yi1 = np.clip(y0f + 1, 0, H - 1).astype(np.int32)
    bn = (np.arange(N * S) // S)[:, None, None]
    out = np.empty((N * S, C, H, W), np.float32)
    for corner, (yi, xi, w) in enumerate(
            [(yi0, xi0, wA), (yi1, xi0, wB), (yi0, xi1, wC), (yi1, xi1, wD)]):
        v = x[bn, :, yi, xi]                                   # [B,H,W,C]
        v = np.moveaxis(v, -1, 1) * w[:, None]
        out = v if corner == 0 else out + v
    return out.reshape(N, S, C, H, W).astype(np.float32)


def _build_nc():
    import sys
    if '/opt/trn_rl_repo' not in sys.path:
        sys.path.insert(0, '/opt/trn_rl_repo')
    import concourse.bacc as bacc
    import concourse.mybir as mybir
    from concourse.library_config import mlp

    nc = bacc.Bacc("TRN2", target_bir_lowering=False, debug=False, num_devices=8)
    PP = nc.dram_tensor("PP", [C, NR, 704], mybir.dt.bfloat16, kind="ExternalInput")
    IX = nc.dram_tensor("IX", [16, QT * QPX // 16], mybir.dt.int16,
                        kind="ExternalInput")
    WF = nc.dram_tensor("WF", [QT * QP, QJ * 5], mybir.dt.bfloat16,
                        kind="ExternalInput")
    P = nc.dram_tensor("P", [GROW, GE], mybir.dt.bfloat16, kind="Internal")
    OUT = nc.dram_tensor("OUT", [QT * QP, C * QJ], mybir.dt.bfloat16,
                         kind="ExternalOutput")

    with (
        nc.Block() as block,
        nc.sbuf_tensor("dst", [QP, 2, QJ, GE], mybir.dt.bfloat16) as dst,
        nc.sbuf_tensor("idx", [QP, 2, PHW], mybir.dt.int16) as idx,
        nc.sbuf_tensor("w", [QP, 2, QJ * 5], mybir.dt.bfloat16) as w,
        nc.sbuf_tensor("wlo", [QP, QJ, 4], mybir.dt.bfloat16) as wlo,
        nc.sbuf_tensor("whi", [QP, QJ, 4], mybir.dt.bfloat16) as whi,
        nc.sbuf_tensor("fp", [QP, QJ], mybir.dt.bfloat16) as fp,
        nc.sbuf_tensor("s1", [QP, 2, QJ, 4], mybir.dt.float32) as s1,
        nc.sbuf_tensor("s2", [QP, 2, QJ, 4], mybir.dt.float32) as s2,
        nc.sbuf_tensor("ob", [QP, C * QJ], mybir.dt.float32) as ob,
        nc.sbuf_tensor("obb", [QP, 2, C * QJ], mybir.dt.bfloat16) as obb,
        nc.semaphore("sP") as sP,
        nc.semaphore("sI0") as sI0,
        nc.semaphore("sI1") as sI1,
        nc.semaphore("sW0") as sW0,
        nc.semaphore("sW1") as sW1,
        nc.semaphore("sG0") as sG0,
        nc.semaphore("sG1") as sG1,
        nc.semaphore("sB") as sB,
        nc.semaphore("sO0") as sO0,
        nc.semaphore("sO1") as sO1,
    ):
        # DMA completions are NOT ordered across in-flight transfers, so a
        # single cumulative sem per stage can be satisfied by tile t+1's fast
        # DMAs while tile t's slow one is still landing. Per-slot sems make
        # each threshold count exactly the tiles sharing that buffer slot.
        sI = [sI0, sI1]
        sW = [sW0, sW1]
        sG = [sG0, sG1]
        sO = [sO0, sO1]
        @block.sync
        def _(sync):
            for k in range(NK):
                off = 128 - 2 * k
                src = PP[:, :, off:off + 512].rearrange("c r (b e) -> c r b e", b=NB)
                dstap = P[k * NR * NB:(k + 1) * NR * NB, :].rearrange(
                    "(r b) (c e) -> c r b e", b=NB, c=C)
                sync.dma_start(dstap, src).then_inc(sP, 16)
            for ph in range(2):
                for g in range(8):
                    sync.dma_start(idx[16 * g:16 * g + 16, ph, :],
                                   IX[:, ph * PHW:(ph + 1) * PHW]).then_inc(sI[ph], 16)
            for t in range(QT):
                if t >= 2:
                    sync.wait_ge(sB, t - 1)
                # refill idx phase ph at t=(ph-1)*PHQ+1: the sB wait above
                # implies all phase-(ph-2) tiles are gathered (slot free)
                if t >= PHQ + 1 and (t - 1) % PHQ == 0:
                    ph = (t - 1) // PHQ + 1
                    if ph < NPH:
                        for g in range(8):
                            sync.dma_start(idx[16 * g:16 * g + 16, ph % 2, :],
                                           IX[:, ph * PHW:(ph + 1) * PHW]
                                           ).then_inc(sI[ph % 2], 16)
                sync.dma_start(
                    w[:, t % 2, :], WF[t * QP:(t + 1) * QP, :]
                ).then_inc(sW[t % 2], 16)
            for z in range(2):
                sync.wait_ge(sI[z], 16 * 8 * (NPH // 2))
                sync.wait_ge(sW[z], 16 * (QT // 2))
            sync.wait_ge(sP, 16 * NK)

        @block.gpsimd
        def _(gp):
            gp.load_library(mlp)
            gp.wait_ge(sP, 16 * NK)
            for t in range(QT):
                ph = t // PHQ
                gp.wait_ge(sI[ph % 2], 16 * 8 * (ph // 2 + 1))
                if t >= 2:
                    gp.wait_ge(sB, t - 1)
                for c8 in range(NCALL):
                    col = (t % PHQ) * (QPX // 16) + c8 * (CALLI // 16)
                    gp.dma_gather(
                        dst[:, t % 2, 8 * c8:8 * c8 + 8, :], P[:, :],
                        idx[:, ph % 2, col:col + CALLI // 16],
                        CALLI, CALLI, GE).then_inc(sG[t % 2], 16)
            for z in range(2):
                gp.wait_ge(sG[z], 16 * NCALL * (QT // 2))

        @block.vector
        def _(ve):
            for t in range(QT):
                ve.wait_ge(sW[t % 2], 16 * (t // 2 + 1))
                ve.wait_ge(sG[t % 2], 16 * NCALL * (t // 2 + 1))
                if t >= 2:
                    ve.wait_ge(sO[t % 2], 16 * (t // 2))
                z = t % 2
                w4v = w[:, z, 0:QJ * 4].rearrange("p (j o) -> p j o", o=4)
                fv = w[:, z, QJ * 4:QJ * 5]
                fb = fv.unsqueeze(-1).broadcast_to((QP, QJ, 4))
                ve.tensor_tensor(out=whi[:, :, :], in0=w4v, in1=fb,
                                 op=mybir.AluOpType.mult)
                ve.tensor_tensor(out=wlo[:, :, :], in0=w4v, in1=whi[:, :, :],
                                 op=mybir.AluOpType.subtract)
                for c in range(C):
                    glo = dst[:, z, :, 128 * c:128 * c + 4]
                    ghi = dst[:, z, :, 128 * c + 64:128 * c + 68]
                    ve.tensor_tensor(out=s1[:, z, :, :], in0=glo, in1=wlo[:, :, :],
                                     op=mybir.AluOpType.mult)
                    ve.tensor_tensor(out=s2[:, z, :, :], in0=ghi, in1=whi[:, :, :],
                                     op=mybir.AluOpType.mult)
                    ve.tensor_tensor(out=s1[:, z, :, :], in0=s1[:, z, :, :],
                                     in1=s2[:, z, :, :], op=mybir.AluOpType.add)
                    ve.tensor_reduce(out=ob[:, QJ * c:QJ * c + QJ],
                                     in_=s1[:, z, :, :],
                                     axis=mybir.AxisListType.X,
                                     op=mybir.AluOpType.add)
                ve.tensor_copy(out=obb[:, z, :], in_=ob[:, :]).then_inc(sB, 1)

        @block.scalar
        def _(sc):
            for t in range(QT):
                sc.wait_ge(sB, t + 1)
                sc.dma_start(OUT[t * QP:(t + 1) * QP, :],
                             obb[:, t % 2, :]).then_inc(sO[t % 2], 16)
            for z in range(2):
                sc.wait_ge(sO[z], 16 * (QT // 2))

    nc.compile()
    return nc


def kernel(x, ksamp, rot_factor):
    import sys, time
    x = np.asarray(x, dtype=np.float32)
    ksamp = np.asarray(ksamp, dtype=np.float32)
    rot_factor = np.asarray(rot_factor, dtype=np.float32)
    try:
        PPs, IXs, WFs = _host_precompute(x, ksamp, rot_factor)
        nc = _build_nc()
        if '/opt/trn_rl_repo' not in sys.path:
            sys.path.insert(0, '/opt/trn_rl_repo')
        from concourse.bass_utils import run_bass_kernel_spmd
        in_maps = [{"PP": PPs[n], "IX": IXs[n], "WF": WFs[n]} for n in range(N)]
        res = None
        for attempt in range(3):
            try:
                res = run_bass_kernel_spmd(nc, in_maps, core_ids=list(range(8)))
                break
            except Exception as e:
                print(f"kernel.py: device attempt {attempt} failed "
                      f"({type(e).__name__}); retrying", file=sys.stderr)
                time.sleep(5)
        if res is None:
            raise RuntimeError("device retries exhausted")
        out = np.empty((N, S, C, H, W), np.float32)
        for n in range(N):
            out[n] = _reorder_out(res.results[n]["OUT"])
        return out
    except Exception as e:
        print(f"kernel.py: device path failed ({type(e).__name__}: {e}); "
              f"using numpy fallback", file=sys.stderr)
        return _numpy_fallback(x, ksamp, rot_factor)
